# revision 1
# baseline (speedup 1.0000x reference)
"""HMQ-quantized MLP (fc1 -> exact GELU -> fc2) on 8 TRN2 NeuronCores.

Strategy: data-parallel over the 16384 token rows (2048 rows/core).
The int8 fake-quant values are integers in [-127, 127], exactly representable
in bf16, and all dot-product partial sums stay far below 2^24 -- so the
dequantized GEMMs are computed EXACTLY as bf16 integer matmuls on the PE
array with fp32 PSUM accumulation, then scaled by s_a*s_w.  Rounding uses
the +/-1.5*2^23 magic-constant trick (matches jnp.round round-half-even).

All quantization reads f32 sources (any bf16 pre-rounding of continuous
values flips ~3% of the round-to-int decisions and costs ~1% rel-err per
tensor -- measured, not acceptable).  g stages to DRAM in f32.

Scheduling: no on-device transposes (b1 pre-blocked on host); phase-A
abs-max pass uses 1MB chunks so DVE reduces keep up with the 358 GB/s DMA
stream; the x re-read for quantization is issued during the scale
AllGather; x quantize alternates Scalar/Vector engines in matmul
consumption order; w1 batches are quantized one batch ahead of use so the
strict-FIFO Scalar queue never stalls the next LDWEIGHTS; collective
doorbells ride an empty DMA queue ahead of bulk traffic.
"""

import numpy as np

import concourse.bass as bass
import concourse.mybir as mybir
import concourse.tile as tile
from concourse import bacc, bass_isa
from concourse.bass_utils import run_bass_kernel_spmd

F32 = mybir.dt.float32
BF16 = mybir.dt.bfloat16
ts = bass.ts

C_MAGIC = 1.5 * 2**23  # round-to-nearest-even for |v| < 2^22
QMAX = 127.0

NCORES = 8
B, T, D, H = 4, 4096, 1024, 4096
M = B * T            # 16384 total rows
S = M // NCORES      # 2048 rows per core

N_IC = D // 128      # 8  contraction chunks for fc1
N_OC = H // 128      # 32 output chunks for fc1 (hidden)
N_ST = S // 512      # 4  row tiles of 512
N_SC = S // 128      # 16 row chunks of 128
N_NC = H // 128      # 32 contraction chunks for fc2
N_JT = D // 512      # 2  output col tiles for fc2

Copy = mybir.ActivationFunctionType.Copy
Gelu = mybir.ActivationFunctionType.Gelu
X_AX = mybir.AxisListType.X
MAX = mybir.AluOpType.max
MULT = mybir.AluOpType.mult
SUB = mybir.AluOpType.subtract
ADD = mybir.AluOpType.add


def build():
    nc = bacc.Bacc("TRN2", target_bir_lowering=False, debug=False,
                   num_devices=NCORES)

    xts = nc.dram_tensor("xts", [D, S], F32, kind="ExternalInput")
    w1r = nc.dram_tensor("w1r", [128, N_IC, H], F32, kind="ExternalInput")
    w1s = nc.dram_tensor("w1s", [H // NCORES, D], F32, kind="ExternalInput")
    w2t = nc.dram_tensor("w2t", [H, D], F32, kind="ExternalInput")
    w2s = nc.dram_tensor("w2s", [D // NCORES, H], F32, kind="ExternalInput")
    b1a = nc.dram_tensor("b1a", [128, N_OC], F32, kind="ExternalInput")
    b2m = nc.dram_tensor("b2m", [128, D], F32, kind="ExternalInput")
    out = nc.dram_tensor("out", [S, D], F32, kind="ExternalOutput")

    with tile.TileContext(nc) as tc:
        with (
            tc.tile_pool(name="misc", bufs=1) as misc,
            tc.tile_pool(name="fs", bufs=4) as fsp,
            tc.tile_pool(name="xq", bufs=1) as xqp,
            tc.tile_pool(name="w1c", bufs=3) as w1sp,
            tc.tile_pool(name="w1q", bufs=2) as w1qp,
            tc.tile_pool(name="w2c", bufs=1) as w2sp,
            tc.tile_pool(name="w2q", bufs=1) as w2qp,
            tc.tile_pool(name="gout", bufs=4) as goutp,
            tc.tile_pool(name="gts", bufs=2) as gtsp,
            tc.tile_pool(name="gq", bufs=2) as gqp,
            tc.tile_pool(name="outp", bufs=2) as outp,
            tc.tile_pool(name="psum", bufs=8, space="PSUM") as psump,
            tc.tile_pool(name="dram", bufs=1, space="DRAM") as dramp,
        ):
            # ---------------- persistent DRAM intermediates ----------------
            gT = dramp.tile([H, S], F32, tag="gT")
            cc1_in = dramp.tile([1, 4], F32, tag="cc1i")
            cc1_out = dramp.tile([NCORES, 4], F32, tag="cc1o")
            cc2_in = dramp.tile([1, 4], F32, tag="cc2i")
            cc2_out = dramp.tile([NCORES, 4], F32, tag="cc2o")

            # ---------------- bias prep (no transposes needed) -------------
            b1sb = misc.tile([128, N_OC], F32, tag="b1sb")
            nc.sync.dma_start(out=b1sb, in_=b1a[:, :])
            b2r = misc.tile([128, D], F32, tag="b2r")
            nc.sync.dma_start(out=b2r, in_=b2m[:, :])

            # ------------- local abs-max pass (1MB chunks) -----------------
            # part1 cols: 0..7 x | 8..9 w1 | 10..11 w2
            part1 = misc.tile([128, 12], F32, tag="part1")
            for ic in range(N_IC):
                xc = fsp.tile([128, 2048], F32, tag="fs", name=f"xmax{ic}")
                nc.sync.dma_start(out=xc, in_=xts[ic * 128:(ic + 1) * 128, :])
                nc.vector.tensor_reduce(out=part1[:, ic:ic + 1], in_=xc,
                                        axis=X_AX,
                                        op=MAX, apply_absolute_value=True)
            # w1 shard [512, 1024] -> 2 chunks [128, 2, 1024]
            for c in range(2):
                wc = fsp.tile([128, 2, 1024], F32, tag="fs", name=f"w1m{c}")
                nc.sync.dma_start(
                    out=wc,
                    in_=w1s[c * 256:(c + 1) * 256, :].rearrange(
                        "(a p) d -> p a d", p=128))
                nc.vector.tensor_reduce(out=part1[:, 8 + c:9 + c], in_=wc,
                                        axis=mybir.AxisListType.XY,
                                        op=MAX, apply_absolute_value=True)
            # w2 shard [128, 4096] -> 2 chunks [128, 2048]
            for c in range(2):
                wc = fsp.tile([128, 2048], F32, tag="fs", name=f"w2m{c}")
                nc.sync.dma_start(out=wc,
                                  in_=w2s[:, c * 2048:(c + 1) * 2048])
                nc.vector.tensor_reduce(out=part1[:, 10 + c:11 + c], in_=wc,
                                        axis=X_AX,
                                        op=MAX, apply_absolute_value=True)

            # combine partials -> [x, w1, w2, w2] cols of arow
            arow = misc.tile([128, 4], F32, tag="arow")
            nc.vector.tensor_reduce(out=arow[:, 0:1], in_=part1[:, 0:8],
                                    axis=X_AX, op=MAX)
            nc.vector.tensor_reduce(out=arow[:, 1:2], in_=part1[:, 8:10],
                                    axis=X_AX, op=MAX)
            nc.vector.tensor_reduce(out=arow[:, 2:3], in_=part1[:, 10:12],
                                    axis=X_AX, op=MAX)
            nc.vector.tensor_copy(arow[:, 3:4], arow[:, 2:3])
            armax = misc.tile([128, 4], F32, tag="armax")
            nc.gpsimd.partition_all_reduce(armax, arow, channels=128,
                                           reduce_op=bass_isa.ReduceOp.max)

            # ------------- AllGather #1 -> global Mx, Mw1, Mw2 -------------
            nc.gpsimd.dma_start(out=cc1_in, in_=armax[0:1, :])
            nc.gpsimd.collective_compute(
                "AllGather", mybir.AluOpType.bypass,
                replica_groups=[list(range(NCORES))],
                ins=[cc1_in.opt()], outs=[cc1_out.opt()])

            # w1 prefetch for the first two batches (sync queue); the x
            # re-read rides the GPSIMD queue BEHIND the collective doorbell
            # so the doorbell's tiny DRAM write never queues behind bulk
            # traffic, while the re-read still overlaps the collective.
            # ALL startup prefetch rides the GPSIMD queue BEHIND the
            # collective doorbell (same queue, later position): the 16B
            # trigger write completes on quiet HBM, and the prefetch then
            # overlaps the collective. Order: w1 batches + first x chunks
            # first (fc1's first matmuls need them), then the rest.
            w1cs = []
            xcs = [None] * N_IC
            w2pre = {}
            with tc.tile_wait_until(0.055):
                for ocb in range(2):
                    w1c = w1sp.tile([128, N_IC, 256], F32, tag="w1c",
                                    name=f"w1c{ocb}")
                    nc.gpsimd.dma_start(
                        out=w1c, in_=w1r[:, :, ocb * 256:(ocb + 1) * 256])
                    w1cs.append(w1c)
                for ic in range(N_IC):
                    xc = fsp.tile([128, 2048], F32, tag="fs",
                                  name=f"xrd{ic}")
                    nc.gpsimd.dma_start(out=xc,
                                        in_=xts[ic * 128:(ic + 1) * 128, :])
                    xcs[ic] = xc
                w2c0 = w2sp.tile([128, D], F32, tag="w2c", name="w2c0")
                nc.gpsimd.dma_start(out=w2c0, in_=w2t[0:128, :])
                w2pre[0] = w2c0

            g1g = misc.tile([NCORES, 4], F32, tag="g1g")
            nc.gpsimd.dma_start(out=g1g, in_=cc1_out[:, :])
            g1m = misc.tile([NCORES, 4], F32, tag="g1m")
            nc.gpsimd.partition_all_reduce(g1m, g1g, channels=NCORES,
                                           reduce_op=bass_isa.ReduceOp.max)
            g1 = misc.tile([128, 4], F32, tag="g1")
            nc.gpsimd.partition_broadcast(g1, g1m)

            # scl cols: 0 sx | 1 inv_sx | 2 sw1 | 3 inv_sw1 | 4 sw2 |
            #           5 inv_sw2 | 6 d1
            scl = misc.tile([128, 8], F32, tag="scl")
            for i in range(3):
                nc.vector.tensor_scalar(out=scl[:, 2 * i:2 * i + 1],
                                        in0=g1[:, i:i + 1],
                                        scalar1=1e-8, scalar2=1.0 / QMAX,
                                        op0=MAX, op1=MULT)
                nc.vector.reciprocal(scl[:, 2 * i + 1:2 * i + 2],
                                     scl[:, 2 * i:2 * i + 1])
            nc.vector.tensor_mul(scl[:, 6:7], scl[:, 0:1], scl[:, 2:3])

            def quant_w1(ocb):
                # DVE-only quantize: keeps the strict-FIFO Scalar queue free
                # so gelus drain PSUM banks on time (PE stalls otherwise)
                w1c = w1cs[ocb]
                w1q = w1qp.tile([128, N_IC, 256], BF16, tag="w1q",
                                name=f"w1q{ocb}")
                w1cf = w1c.rearrange("p a b -> p (a b)")
                nc.vector.tensor_scalar(out=w1cf, in0=w1cf,
                                        scalar1=scl[:, 3:4], scalar2=C_MAGIC,
                                        op0=MULT, op1=ADD)
                nc.vector.tensor_scalar(
                    out=w1q.rearrange("p a b -> p (a b)"), in0=w1cf,
                    scalar1=C_MAGIC, scalar2=None, op0=SUB)
                return w1q

            # batch-0 weights quantize FIRST so fc1 can start on xq chunk 0
            w1qs = [quant_w1(0)]

            # -------- quantize x -> xqT bf16, split across Scalar/Vector ---
            xqT = xqp.tile([128, N_IC, S], BF16, tag="xq")
            # quantize in 16 half-chunks, alternating engines per half so
            # each ic becomes ready ~1.5us apart (fc1's oc0 consumes an ic
            # every ~0.9us; coarse chunks left the PE waiting ~15us)
            for ic in range(N_IC):
                xc = xcs[ic]
                for hf in range(2):
                    xh = xc[:, hf * 1024:(hf + 1) * 1024]
                    qh = xqT[:, ic, hf * 1024:(hf + 1) * 1024]
                    if hf == 0:
                        nc.scalar.activation(xh, xh, Copy,
                                             bias=C_MAGIC, scale=scl[:, 1:2])
                        nc.scalar.activation(qh, xh, Copy,
                                             bias=-C_MAGIC, scale=1.0)
                    else:
                        nc.vector.tensor_scalar(out=xh, in0=xh,
                                                scalar1=scl[:, 1:2],
                                                scalar2=C_MAGIC,
                                                op0=MULT, op1=ADD)
                        nc.vector.tensor_scalar(out=qh, in0=xh,
                                                scalar1=C_MAGIC, scalar2=None,
                                                op0=SUB)

            # ---------------- fc1: h^T = w1q @ xq^T, gelu, stage g^T -------
            # st-outer / ic-inner so consecutive matmuls load different
            # weights (background weight loads stay hidden).
            w2qT = w2qp.tile([128, N_NC, D], BF16, tag="w2q")
            gpart = misc.tile([128, N_OC * N_ST], F32, tag="gpart")
            for ocb in range(N_OC // 2):
                # prefetch batch k+2's w1 and quantize batch k+1 at the
                # TOP of iteration k: the quant Scalar op then sits AHEAD
                # of this batch's gelu ACTs in the strict-FIFO Scalar queue
                # and completes a full window before its LDWEIGHTS need it.
                if ocb + 2 <= N_OC // 2 - 1:
                    w1n = w1sp.tile([128, N_IC, 256], F32, tag="w1c",
                                    name=f"w1c{ocb + 2}")
                    nc.sync.dma_start(
                        out=w1n,
                        in_=w1r[:, :, (ocb + 2) * 256:(ocb + 3) * 256])
                    w1cs.append(w1n)
                if ocb + 1 <= N_OC // 2 - 1:
                    w1qs.append(quant_w1(ocb + 1))
                # w2 prefetch at the top of the window: the DVE subs then
                # precede this window's absmaxes and never crowd the cc2
                # trigger's garow chain at fc1's end
                if ocb >= 1:
                    nq = 4 if ocb == N_OC // 2 - 1 else 2
                    for q in range(nq):
                        hc = 2 * (ocb - 1) + q
                        if hc in w2pre:
                            w2c = w2pre[hc]
                        else:
                            w2c = w2sp.tile([128, D], F32, tag="w2c",
                                            name=f"w2c{hc}")
                            nc.sync.dma_start(out=w2c,
                                              in_=w2t[ts(hc, 128), :])
                        nc.scalar.activation(w2c, w2c, Copy, bias=C_MAGIC,
                                             scale=scl[:, 5:6])
                        nc.vector.tensor_scalar(out=w2qT[:, hc, :], in0=w2c,
                                                scalar1=C_MAGIC,
                                                scalar2=None, op0=SUB)
                w1q = w1qs[ocb]
                for j in range(2):
                    oc = 2 * ocb + j
                    pts = [psump.tile([128, 512], F32, tag="mm",
                                      name=f"pt{oc}_{st}")
                           for st in range(N_ST)]
                    for ic in range(N_IC):
                        for st in range(N_ST):
                            nc.tensor.matmul(
                                pts[st],
                                lhsT=w1q[:, ic, j * 128:(j + 1) * 128],
                                rhs=xqT[:, ic, ts(st, 512)],
                                start=(ic == 0), stop=(ic == N_IC - 1))
                    for st in range(N_ST):
                        go = goutp.tile([128, 512], F32, tag="gout",
                                        name=f"go{oc}_{st}")
                        nc.scalar.activation(go, pts[st], Gelu,
                                             bias=b1sb[:, oc:oc + 1],
                                             scale=scl[:, 6:7])
                        nc.vector.tensor_reduce(
                            out=gpart[:, oc * N_ST + st:oc * N_ST + st + 1],
                            in_=go, axis=X_AX, op=MAX,
                            apply_absolute_value=True)
                        nc.sync.dma_start(out=gT[ts(oc, 128), ts(st, 512)],
                                          in_=go)

            # ---------------- AllGather #2 trigger: global Mg --------------
            garow = misc.tile([128, 4], F32, tag="garow")
            nc.vector.tensor_reduce(out=garow[:, 0:1], in_=gpart, axis=X_AX,
                                    op=MAX)
            for jj in range(1, 4):
                nc.vector.tensor_copy(garow[:, jj:jj + 1], garow[:, 0:1])
            gamax = misc.tile([128, 4], F32, tag="gamax")
            nc.gpsimd.partition_all_reduce(gamax, garow, channels=128,
                                           reduce_op=bass_isa.ReduceOp.max)
            nc.gpsimd.dma_start(out=cc2_in, in_=gamax[0:1, :])
            nc.gpsimd.collective_compute(
                "AllGather", mybir.AluOpType.bypass,
                replica_groups=[list(range(NCORES))],
                ins=[cc2_in.opt()], outs=[cc2_out.opt()])

            g2g = misc.tile([NCORES, 4], F32, tag="g2g")
            nc.gpsimd.dma_start(out=g2g, in_=cc2_out[:, :])
            g2m = misc.tile([NCORES, 4], F32, tag="g2m")
            nc.gpsimd.partition_all_reduce(g2m, g2g, channels=NCORES,
                                           reduce_op=bass_isa.ReduceOp.max)
            g2 = misc.tile([128, 4], F32, tag="g2")
            nc.gpsimd.partition_broadcast(g2, g2m)

            # scl2 cols: 0 sg | 1 inv_sg | 2 d2
            scl2 = misc.tile([128, 4], F32, tag="scl2")
            nc.vector.tensor_scalar(out=scl2[:, 0:1], in0=g2[:, 0:1],
                                    scalar1=1e-8, scalar2=1.0 / QMAX,
                                    op0=MAX, op1=MULT)
            nc.vector.reciprocal(scl2[:, 1:2], scl2[:, 0:1])
            nc.vector.tensor_mul(scl2[:, 2:3], scl2[:, 0:1], scl[:, 4:5])

            # ---------------- fc2: out = gq^T.T @ w2q^T --------------------
            # jt-outer / nc-inner: consecutive matmuls load different
            # weights.
            for sc in range(N_SC):
                gqs = []
                for half in range(2):
                    gs = gtsp.tile([128, 16, 128], F32, tag="gts",
                                   name=f"gs{sc}_{half}")
                    nc.sync.dma_start(
                        out=gs,
                        in_=gT[half * 2048:(half + 1) * 2048,
                               ts(sc, 128)].rearrange("(a p) s -> p a s",
                                                      p=128))
                    gq = gqp.tile([128, 16, 128], BF16, tag="gq",
                                  name=f"gq{sc}_{half}")
                    gsf = gs.rearrange("p a b -> p (a b)")
                    nc.scalar.activation(gsf, gsf, Copy, bias=C_MAGIC,
                                         scale=scl2[:, 1:2])
                    nc.vector.tensor_scalar(
                        out=gq.rearrange("p a b -> p (a b)"), in0=gsf,
                        scalar1=C_MAGIC, scalar2=None, op0=SUB)
                    gqs.append(gq)
                pos = [psump.tile([128, 512], F32, tag="mm",
                                  name=f"po{sc}_{jt}")
                       for jt in range(N_JT)]
                for nn in range(N_NC):
                    for jt in range(N_JT):
                        nc.tensor.matmul(pos[jt],
                                         lhsT=gqs[nn // 16][:, nn % 16, :],
                                         rhs=w2qT[:, nn, ts(jt, 512)],
                                         start=(nn == 0),
                                         stop=(nn == N_NC - 1))
                for jt in range(N_JT):
                    ot = outp.tile([128, 512], F32, tag="ot",
                                   name=f"ot{sc}_{jt}")
                    nc.scalar.activation(ot, pos[jt], Copy, bias=0.0,
                                         scale=scl2[:, 2:3])
                    nc.vector.tensor_add(ot, ot, b2r[:, ts(jt, 512)])
                    nc.sync.dma_start(out=out[ts(sc, 128), ts(jt, 512)],
                                      in_=ot)

    nc.compile()
    _dedup_ldweights(nc)
    return nc


def _dedup_ldweights(nc):
    """Remove back-to-back InstLdweights that reload the exact same weights.

    bass emits one LDWEIGHTS per matmul; within an accumulation group that
    shares the stationary operand the reloads are redundant and the HW pays
    ~108ns each (partially exposed in the matmul issue stream).  Deleting a
    reload is safe when it carries no semaphore waits/updates: the matmuls
    still increment the PE completion semaphore, so every WAR threshold
    computed by the tile scheduler is unchanged.
    """
    removed = 0
    for blk in nc.main_func.blocks:
        last_sig = None
        to_remove = []
        for ins in blk.instructions:
            t = type(ins).__name__
            if t == "InstLdweights":
                si = ins.sync_info
                has_sync = si is not None and (list(si.on_wait)
                                               or list(si.on_update))
                sig = (str(ins.ins[0]), str(ins.perf_mode),
                       str(ins.is_transpose))
                if sig == last_sig and not has_sync:
                    to_remove.append(ins)
                else:
                    last_sig = sig
            elif t == "InstMatmult" and ins.is_transpose:
                last_sig = None
        for ins in to_remove:
            blk.instructions.remove(ins)
        removed += len(to_remove)
    return removed


_NC_CACHE = None


def _get_nc():
    global _NC_CACHE
    if _NC_CACHE is None:
        _NC_CACHE = build()
    return _NC_CACHE


def make_in_maps(x, w1, b1, w2, b2):
    xf = np.ascontiguousarray(x.reshape(M, D).T)          # [D, M]
    # w1r[p, ic, h] = w1[h, ic*128+p]
    w1r_h = np.ascontiguousarray(w1.T.reshape(N_IC, 128, H).transpose(1, 0, 2))
    w2t_h = np.ascontiguousarray(w2.T)                    # [H, D]
    b1a_h = np.ascontiguousarray(b1.reshape(N_OC, 128).T)  # [128, 32]
    b2m_h = np.ascontiguousarray(np.broadcast_to(b2.reshape(1, D),
                                               (128, D)))
    in_maps = []
    for c in range(NCORES):
        in_maps.append({
            "xts": np.ascontiguousarray(xf[:, c * S:(c + 1) * S]),
            "w1r": w1r_h,
            "w1s": np.ascontiguousarray(
                w1[c * (H // NCORES):(c + 1) * (H // NCORES), :]),
            "w2t": w2t_h,
            "w2s": np.ascontiguousarray(
                w2[c * (D // NCORES):(c + 1) * (D // NCORES), :]),
            "b1a": b1a_h,
            "b2m": b2m_h,
        })
    return in_maps


def kernel(x, w1, b1, w2, b2, _trace=False):
    nc = _get_nc()
    in_maps = make_in_maps(np.asarray(x, dtype=np.float32),
                           np.asarray(w1, dtype=np.float32),
                           np.asarray(b1, dtype=np.float32),
                           np.asarray(w2, dtype=np.float32),
                           np.asarray(b2, dtype=np.float32))
    res = run_bass_kernel_spmd(nc, in_maps, core_ids=list(range(NCORES)),
                               trace=_trace)
    full = np.concatenate([res.results[c]["out"] for c in range(NCORES)],
                          axis=0)
    out = full.reshape(B, T, D)
    if _trace:
        kernel.last_results = res
    return out

